# revision 3
# baseline (speedup 1.0000x reference)
"""Depthwise 9x9 same-padding conv (single shared kernel) on Trainium2.

Strategy: same banded-Toeplitz PE formulation as kernel_base, plus:
  - PE warmup: junk matmuls on a memset tile ramp the tensor engine's
    p-state while the first input/weight DMAs are in flight, so real
    matmuls all run at the full 2.4 GHz rate.
  - Weights ride the ACT HWDGE ring (idle at start), host-pretransposed
    to [p, v, m] so each is one fast contiguous descriptor per partition.
  - Edge-strip work is emitted before the main half-tiles of its block,
    and the final block streams output per-image to shrink the tail.
"""

import numpy as np

import concourse.bass as bass
from concourse import bacc
import concourse.mybir as mybir
import concourse.tile as tile
from concourse.bass_utils import run_bass_kernel_spmd

N_CORES = 8
B, C, H, W = 32, 64, 256, 256
KS, PAD = 9, 4
BC = B // N_CORES          # batches per core
NIMG = BC * C              # images per core
WP = W + 2 * PAD           # padded width 264
MT = 124                   # main out-rows per half-tile (0..123 / 132..255)
EG = 8                     # images per edge-strip group
NGRP = NIMG // EG
J = 4                      # images packed per main DMA / SBUF tile
NBLK = NIMG // J

IN_DT = mybir.dt.float32r
IN_NP = np.float32

LAST_RESULT = None         # test.py inspects this


def _build_weights(Kf):
    """Banded Toeplitz lhsT matrices from the 9x9 kernel Kf (float32),
    pretransposed to [p, v, m] host-side so the DMA is contiguous."""
    Wtop = np.zeros((KS, 128, MT), np.float32)
    Wbot = np.zeros((KS, 128, MT), np.float32)
    Wedge = np.zeros((KS, 128, 8 * EG), np.float32)
    for v in range(KS):
        for i in range(MT):
            for u in range(KS):
                ip = i + u - PAD
                if 0 <= ip < 128:
                    Wtop[v, ip, i] = Kf[u, v]
                ipb = i + u
                if 0 <= ipb < 128:
                    Wbot[v, ipb, i] = Kf[u, v]
        for g in range(EG):
            for m in range(8):
                for u in range(KS):
                    Wedge[v, 16 * g + m + u, 8 * g + m] = Kf[u, v]
    tr = lambda w: np.ascontiguousarray(w.transpose(1, 0, 2))
    return tr(Wtop), tr(Wbot), tr(Wedge)


def _build_nc(n_img=NIMG, xbufs=4, obufs=4, psbufs=5, nwarm=6, gate=True,
              psum_tail=False):
    n_blk = n_img // J
    n_grp = n_img // EG
    nc = bacc.Bacc("TRN2", target_bir_lowering=False)
    Xm = nc.dram_tensor("Xm", [n_blk, 2, 128, J * WP], IN_DT, kind="ExternalInput")
    Xe = nc.dram_tensor("Xe", [n_grp, 128, WP], IN_DT, kind="ExternalInput")
    Wt = nc.dram_tensor("Wt", [128, KS, MT], IN_DT, kind="ExternalInput")
    Wb = nc.dram_tensor("Wb", [128, KS, MT], IN_DT, kind="ExternalInput")
    We = nc.dram_tensor("We", [128, KS, 8 * EG], IN_DT, kind="ExternalInput")
    Om = nc.dram_tensor(
        "Om", [n_blk, 2, MT, J * W], mybir.dt.float32, kind="ExternalOutput"
    )
    Oe = nc.dram_tensor(
        "Oe", [n_grp, 8 * EG, W], mybir.dt.float32, kind="ExternalOutput"
    )

    with tile.TileContext(nc) as tc:
        with (
            tc.tile_pool(name="wpool", bufs=1) as wpool,
            tc.tile_pool(name="xpool", bufs=xbufs) as xpool,
            tc.tile_pool(name="epool", bufs=2) as epool,
            tc.tile_pool(name="opool", bufs=obufs) as opool,
            tc.tile_pool(name="oepool", bufs=2) as oepool,
            tc.tile_pool(name="psum", bufs=psbufs, space="PSUM") as pspool,
            tc.tile_pool(name="psum_e", bufs=2, space="PSUM") as pepool,
            tc.tile_pool(name="psum_j", bufs=1, space="PSUM") as pjpool,
        ):
            # PE warmup: junk matmuls on a zeroed tile get the tensor engine
            # busy ASAP and keep it busy until real work lands — the p-state
            # ramp clock starts at first-busy and resets after a long idle.
            if nwarm:
                # bf16: memset on float32r fails the neuronxcc ISA check, and
                # bf16 matmuls stream at the same 1 row/cycle for the warmup
                zt = wpool.tile([128, 512], mybir.dt.bfloat16)
                nc.vector.memset(zt[:], 0.0)
                psj = pjpool.tile([128, 512], mybir.dt.float32)
                for _ in range(nwarm):
                    nc.tensor.matmul(
                        psj[:], zt[:, :128], zt[:], start=True, stop=True
                    )

            # Startup pipeline fill: the DMA engines drain one transfer at a
            # time, so order transfers by when the PE first needs them; the
            # first block's input arrives per-image so j0 can start early.
            VS = 5  # wt arrives in two chunks; v<VS unblocks the first group
            wt_a = wpool.tile([128, VS, MT], IN_DT)
            wt_b = wpool.tile([128, KS - VS, MT], IN_DT)
            wb = wpool.tile([128, KS, MT], IN_DT)
            we = wpool.tile([128, KS, 8 * EG], IN_DT)
            x0 = [xpool.tile([128, WP], IN_DT, name=f"x0_{j}") for j in range(2 * J)]
            nc.sync.dma_start(out=wt_a[:], in_=Wt[:, :VS])
            nc.sync.dma_start(out=wt_b[:], in_=Wt[:, VS:])
            for j in range(2 * J):
                nc.sync.dma_start(
                    out=x0[j][:],
                    in_=Xm[0, j // J].rearrange("p (j w) -> p j w", j=J)[:, j % J],
                )
            if gate:
                # The DMA engines round-robin across the SP/ACT/SWDGE queues
                # and engines dispatch DMAs out of order, so the only way to
                # keep wb/we off the startup wire (where they'd delay the
                # first input tiles) is a real data dependency: a tiny copy
                # into each tile that reads a first-block input tile.
                nc.vector.tensor_copy(wb[:1, 0, :1], x0[0][:1, :1])
                nc.vector.tensor_copy(we[:1, 0, :1], x0[1][:1, :1])
            nc.scalar.dma_start(out=wb[:], in_=Wb[:])
            nc.scalar.dma_start(out=we[:], in_=We[:])
            egate = 2  # first two edge tiles get the same gating treatment

            for blk in range(n_blk):
                for half in range(2):
                    first = blk == 0
                    if not first:
                        xt = xpool.tile([128, J * WP], IN_DT)
                        nc.sync.dma_start(out=xt[:], in_=Xm[blk, half])
                    last = blk == n_blk - 1 and half == 1
                    ot = None if last else opool.tile([MT, J * W], mybir.dt.float32)
                    for j in range(J):
                        ps = pspool.tile([MT, W], mybir.dt.float32)
                        for v in range(KS):
                            if half == 0:
                                wv = wt_a[:, v, :] if v < VS else wt_b[:, v - VS, :]
                            else:
                                wv = wb[:, v, :]
                            rhs = (
                                x0[half * J + j][:, v : v + W]
                                if first
                                else xt[:, j * WP + v : j * WP + v + W]
                            )
                            nc.tensor.matmul(
                                ps[:],
                                wv,
                                rhs,
                                start=(v == 0),
                                stop=(v == KS - 1),
                            )
                        if last and psum_tail:
                            # final tile: skip the SBUF bounce, DMA straight
                            # from PSUM so the tail is one short transfer
                            nc.scalar.dma_start(
                                out=Om[blk, half, :, j * W : (j + 1) * W],
                                in_=ps[:],
                            )
                        elif last:
                            otj = opool.tile([MT, W], mybir.dt.float32)
                            nc.vector.tensor_copy(otj[:], ps[:])
                            nc.scalar.dma_start(
                                out=Om[blk, half, :, j * W : (j + 1) * W],
                                in_=otj[:],
                            )
                        else:
                            nc.vector.tensor_copy(ot[:, j * W : (j + 1) * W], ps[:])
                    if not last:
                        nc.scalar.dma_start(out=Om[blk, half], in_=ot[:])

                # edge groups ride even blocks, shifted one group early so the
                # last group's output DMA isn't the kernel's final straggler:
                # blk0 handles g0+g1, blk 2(g-1) handles g for g>=2
                if blk == 0:
                    gs = [0, 1]
                elif blk % 2 == 0 and blk // 2 + 1 < n_grp:
                    gs = [blk // 2 + 1]
                else:
                    gs = []
                for g in gs:
                    et = epool.tile([128, WP], IN_DT)
                    if gate and egate > 0:
                        egate -= 1
                        nc.vector.tensor_copy(et[:1, :1], x0[2 + egate][:1, :1])
                    nc.gpsimd.dma_start(out=et[:], in_=Xe[g])
                    pse = pepool.tile([8 * EG, W], mybir.dt.float32)
                    for v in range(KS):
                        nc.tensor.matmul(
                            pse[:],
                            we[:, v, :],
                            et[:, v : v + W],
                            start=(v == 0),
                            stop=(v == KS - 1),
                        )
                    oe = oepool.tile([8 * EG, W], mybir.dt.float32)
                    nc.vector.tensor_copy(oe[:], pse[:])
                    nc.gpsimd.dma_start(out=Oe[g], in_=oe[:])
    nc.compile()
    return nc


def _prep_inputs(X):
    """Host prep: pad width, pack J images per tile row-block."""
    Xp = np.zeros((B * C, H, WP), IN_NP)
    Xp[:, :, PAD : PAD + W] = X.reshape(B * C, H, W)
    Xm = (
        Xp.reshape(N_CORES, NBLK, J, 2, 128, WP)
        .transpose(0, 1, 3, 4, 2, 5)
        .reshape(N_CORES, NBLK, 2, 128, J * WP)
    )
    Xm = np.ascontiguousarray(Xm)
    Xe = np.ascontiguousarray(
        Xp[:, 120:136, :].reshape(N_CORES, NGRP, 128, WP)
    )
    return Xm, Xe


def _assemble_output(res):
    """Reassemble [B, C, H, W] fp32 from per-core Om/Oe."""
    out = np.empty((N_CORES, NIMG, H, W), np.float32)
    for k in range(N_CORES):
        om = res.results[k]["Om"].reshape(NBLK, 2, MT, J, W)
        oe = res.results[k]["Oe"].reshape(NGRP * EG, 8, W)
        o = out[k].reshape(NBLK, J, H, W)
        o[:, :, 0:MT, :] = om[:, 0].transpose(0, 2, 1, 3)
        o[:, :, 132 : 132 + MT, :] = om[:, 1].transpose(0, 2, 1, 3)
        out[k][:, 124:132, :] = oe
    return out.reshape(B, C, H, W)


def kernel(X, K):
    global LAST_RESULT
    X = np.asarray(X)
    K = np.asarray(K)
    assert X.shape == (B, C, H, W) and K.shape == (1, 1, KS, KS)

    Xm, Xe = _prep_inputs(X)
    Wtop, Wbot, Wedge = _build_weights(K[0, 0].astype(np.float32))

    nc = _build_nc()
    in_maps = [
        {"Xm": Xm[k], "Xe": Xe[k], "Wt": Wtop, "Wb": Wbot, "We": Wedge}
        for k in range(N_CORES)
    ]
    res = run_bass_kernel_spmd(nc, in_maps, core_ids=list(range(N_CORES)))
    LAST_RESULT = res
    return _assemble_output(res)


# revision 4
# speedup vs baseline: 1.0042x; 1.0042x over previous
"""Depthwise 9x9 same-padding conv (single shared kernel) on Trainium2.

Strategy: same banded-Toeplitz PE formulation as kernel_base, plus:
  - PE warmup: junk matmuls on a memset tile ramp the tensor engine's
    p-state while the first input/weight DMAs are in flight, so real
    matmuls all run at the full 2.4 GHz rate.
  - Weights ride the ACT HWDGE ring (idle at start), host-pretransposed
    to [p, v, m] so each is one fast contiguous descriptor per partition.
  - Edge-strip work is emitted before the main half-tiles of its block,
    and the final block streams output per-image to shrink the tail.
"""

import numpy as np

import concourse.bass as bass
from concourse import bacc
import concourse.mybir as mybir
import concourse.tile as tile
from concourse.bass_utils import run_bass_kernel_spmd

N_CORES = 8
B, C, H, W = 32, 64, 256, 256
KS, PAD = 9, 4
BC = B // N_CORES          # batches per core
NIMG = BC * C              # images per core
WP = W + 2 * PAD           # padded width 264
MT = 124                   # main out-rows per half-tile (0..123 / 132..255)
EG = 8                     # images per edge-strip group
NGRP = NIMG // EG
J = 4                      # images packed per main DMA / SBUF tile
NBLK = NIMG // J

IN_DT = mybir.dt.float32r
IN_NP = np.float32

LAST_RESULT = None         # test.py inspects this


def _build_weights(Kf):
    """Banded Toeplitz lhsT matrices from the 9x9 kernel Kf (float32),
    pretransposed to [p, v, m] host-side so the DMA is contiguous."""
    Wtop = np.zeros((KS, 128, MT), np.float32)
    Wbot = np.zeros((KS, 128, MT), np.float32)
    Wedge = np.zeros((KS, 128, 8 * EG), np.float32)
    for v in range(KS):
        for i in range(MT):
            for u in range(KS):
                ip = i + u - PAD
                if 0 <= ip < 128:
                    Wtop[v, ip, i] = Kf[u, v]
                ipb = i + u
                if 0 <= ipb < 128:
                    Wbot[v, ipb, i] = Kf[u, v]
        for g in range(EG):
            for m in range(8):
                for u in range(KS):
                    Wedge[v, 16 * g + m + u, 8 * g + m] = Kf[u, v]
    tr = lambda w: np.ascontiguousarray(w.transpose(1, 0, 2))
    return tr(Wtop), tr(Wbot), tr(Wedge)


def _build_nc(n_img=NIMG, xbufs=4, obufs=4, psbufs=5, nwarm=6, gate=True,
              psum_tail=False):
    n_blk = n_img // J
    n_grp = n_img // EG
    nc = bacc.Bacc("TRN2", target_bir_lowering=False)
    Xm = nc.dram_tensor("Xm", [n_blk, 2, 128, J * WP], IN_DT, kind="ExternalInput")
    Xe = nc.dram_tensor("Xe", [n_grp, 128, WP], IN_DT, kind="ExternalInput")
    Wt = nc.dram_tensor("Wt", [128, KS, MT], IN_DT, kind="ExternalInput")
    Wb = nc.dram_tensor("Wb", [128, KS, MT], IN_DT, kind="ExternalInput")
    We = nc.dram_tensor("We", [128, KS, 8 * EG], IN_DT, kind="ExternalInput")
    Om = nc.dram_tensor(
        "Om", [n_blk, 2, MT, J * W], mybir.dt.float32, kind="ExternalOutput"
    )
    Oe = nc.dram_tensor(
        "Oe", [n_grp, 8 * EG, W], mybir.dt.float32, kind="ExternalOutput"
    )

    with tile.TileContext(nc) as tc:
        with (
            tc.tile_pool(name="wpool", bufs=1) as wpool,
            tc.tile_pool(name="xpool", bufs=xbufs) as xpool,
            tc.tile_pool(name="epool", bufs=2) as epool,
            tc.tile_pool(name="opool", bufs=obufs) as opool,
            tc.tile_pool(name="oepool", bufs=2) as oepool,
            tc.tile_pool(name="psum", bufs=psbufs, space="PSUM") as pspool,
            tc.tile_pool(name="psum_e", bufs=2, space="PSUM") as pepool,
            tc.tile_pool(name="psum_j", bufs=1, space="PSUM") as pjpool,
        ):
            # PE warmup: junk matmuls on a zeroed tile get the tensor engine
            # busy ASAP and keep it busy until real work lands — the p-state
            # ramp clock starts at first-busy and resets after a long idle.
            if nwarm:
                # bf16: memset on float32r fails the neuronxcc ISA check, and
                # bf16 matmuls stream at the same 1 row/cycle for the warmup
                zt = wpool.tile([128, 512], mybir.dt.bfloat16)
                nc.vector.memset(zt[:], 0.0)
                psj = pjpool.tile([128, 512], mybir.dt.float32)
                for _ in range(nwarm):
                    nc.tensor.matmul(
                        psj[:], zt[:, :128], zt[:], start=True, stop=True
                    )

            # Startup pipeline fill: the DMA engines drain one transfer at a
            # time, so order transfers by when the PE first needs them; the
            # first block's input arrives per-image so j0 can start early.
            VS = 5  # wt arrives in two chunks; v<VS unblocks the first group
            wt_a = wpool.tile([128, VS, MT], IN_DT)
            wt_b = wpool.tile([128, KS - VS, MT], IN_DT)
            wb = wpool.tile([128, KS, MT], IN_DT)
            we = wpool.tile([128, KS, 8 * EG], IN_DT)
            x0 = [xpool.tile([128, WP], IN_DT, name=f"x0_{j}") for j in range(2 * J)]
            nc.sync.dma_start(out=wt_a[:], in_=Wt[:, :VS])
            nc.sync.dma_start(out=wt_b[:], in_=Wt[:, VS:])
            for j in range(2 * J):
                nc.sync.dma_start(
                    out=x0[j][:],
                    in_=Xm[0, j // J].rearrange("p (j w) -> p j w", j=J)[:, j % J],
                )
            if gate:
                # The DMA engines round-robin across the SP/ACT/SWDGE queues
                # and engines dispatch DMAs out of order, so the only way to
                # keep wb/we off the startup wire (where they'd delay the
                # first input tiles) is a real data dependency: a tiny copy
                # into each tile that reads a first-block input tile.
                nc.vector.tensor_copy(wb[:1, 0, :1], x0[0][:1, :1])
                nc.vector.tensor_copy(we[:1, 0, :1], x0[1][:1, :1])
            nc.scalar.dma_start(out=wb[:], in_=Wb[:])
            nc.scalar.dma_start(out=we[:], in_=We[:])
            egate = 2  # first two edge tiles get the same gating treatment

            for blk in range(n_blk):
                for half in range(2):
                    first = blk == 0
                    if not first:
                        xt = xpool.tile([128, J * WP], IN_DT)
                        nc.sync.dma_start(out=xt[:], in_=Xm[blk, half])
                    last = blk == n_blk - 1 and half == 1
                    ot = None if last else opool.tile([MT, J * W], mybir.dt.float32)
                    if not (first or last):
                        # steady state: one N=512 matmul per v covers a pair
                        # of images (2D free AP), halving instruction count
                        xv = xt.rearrange("p (j w) -> p j w", j=J)
                        for jp in range(J // 2):
                            ps = pspool.tile([MT, 2, W], mybir.dt.float32)
                            for v in range(KS):
                                if half == 0:
                                    wv = wt_a[:, v, :] if v < VS else wt_b[:, v - VS, :]
                                else:
                                    wv = wb[:, v, :]
                                nc.tensor.matmul(
                                    ps[:],
                                    wv,
                                    xv[:, 2 * jp : 2 * jp + 2, v : v + W],
                                    start=(v == 0),
                                    stop=(v == KS - 1),
                                )
                            nc.vector.tensor_copy(
                                ot[:, 2 * jp * W : (2 * jp + 2) * W],
                                ps[:].rearrange("p j w -> p (j w)"),
                            )
                        nc.scalar.dma_start(out=Om[blk, half], in_=ot[:])
                        continue
                    for j in range(J):
                        ps = pspool.tile([MT, W], mybir.dt.float32)
                        for v in range(KS):
                            if half == 0:
                                wv = wt_a[:, v, :] if v < VS else wt_b[:, v - VS, :]
                            else:
                                wv = wb[:, v, :]
                            rhs = (
                                x0[half * J + j][:, v : v + W]
                                if first
                                else xt[:, j * WP + v : j * WP + v + W]
                            )
                            nc.tensor.matmul(
                                ps[:],
                                wv,
                                rhs,
                                start=(v == 0),
                                stop=(v == KS - 1),
                            )
                        if last and psum_tail:
                            # final tile: skip the SBUF bounce, DMA straight
                            # from PSUM so the tail is one short transfer
                            nc.scalar.dma_start(
                                out=Om[blk, half, :, j * W : (j + 1) * W],
                                in_=ps[:],
                            )
                        elif last:
                            otj = opool.tile([MT, W], mybir.dt.float32)
                            nc.vector.tensor_copy(otj[:], ps[:])
                            nc.scalar.dma_start(
                                out=Om[blk, half, :, j * W : (j + 1) * W],
                                in_=otj[:],
                            )
                        else:
                            nc.vector.tensor_copy(ot[:, j * W : (j + 1) * W], ps[:])
                    if not last:
                        nc.scalar.dma_start(out=Om[blk, half], in_=ot[:])

                # edge groups ride even blocks, shifted one group early so the
                # last group's output DMA isn't the kernel's final straggler:
                # blk0 handles g0+g1, blk 2(g-1) handles g for g>=2
                if blk == 0:
                    gs = [0, 1]
                elif blk % 2 == 0 and blk // 2 + 1 < n_grp:
                    gs = [blk // 2 + 1]
                else:
                    gs = []
                for g in gs:
                    et = epool.tile([128, WP], IN_DT)
                    if gate and egate > 0:
                        egate -= 1
                        nc.vector.tensor_copy(et[:1, :1], x0[2 + egate][:1, :1])
                    nc.gpsimd.dma_start(out=et[:], in_=Xe[g])
                    pse = pepool.tile([8 * EG, W], mybir.dt.float32)
                    for v in range(KS):
                        nc.tensor.matmul(
                            pse[:],
                            we[:, v, :],
                            et[:, v : v + W],
                            start=(v == 0),
                            stop=(v == KS - 1),
                        )
                    oe = oepool.tile([8 * EG, W], mybir.dt.float32)
                    nc.vector.tensor_copy(oe[:], pse[:])
                    nc.gpsimd.dma_start(out=Oe[g], in_=oe[:])
    nc.compile()
    return nc


def _prep_inputs(X):
    """Host prep: pad width, pack J images per tile row-block."""
    Xp = np.zeros((B * C, H, WP), IN_NP)
    Xp[:, :, PAD : PAD + W] = X.reshape(B * C, H, W)
    Xm = (
        Xp.reshape(N_CORES, NBLK, J, 2, 128, WP)
        .transpose(0, 1, 3, 4, 2, 5)
        .reshape(N_CORES, NBLK, 2, 128, J * WP)
    )
    Xm = np.ascontiguousarray(Xm)
    Xe = np.ascontiguousarray(
        Xp[:, 120:136, :].reshape(N_CORES, NGRP, 128, WP)
    )
    return Xm, Xe


def _assemble_output(res):
    """Reassemble [B, C, H, W] fp32 from per-core Om/Oe."""
    out = np.empty((N_CORES, NIMG, H, W), np.float32)
    for k in range(N_CORES):
        om = res.results[k]["Om"].reshape(NBLK, 2, MT, J, W)
        oe = res.results[k]["Oe"].reshape(NGRP * EG, 8, W)
        o = out[k].reshape(NBLK, J, H, W)
        o[:, :, 0:MT, :] = om[:, 0].transpose(0, 2, 1, 3)
        o[:, :, 132 : 132 + MT, :] = om[:, 1].transpose(0, 2, 1, 3)
        out[k][:, 124:132, :] = oe
    return out.reshape(B, C, H, W)


def kernel(X, K):
    global LAST_RESULT
    X = np.asarray(X)
    K = np.asarray(K)
    assert X.shape == (B, C, H, W) and K.shape == (1, 1, KS, KS)

    Xm, Xe = _prep_inputs(X)
    Wtop, Wbot, Wedge = _build_weights(K[0, 0].astype(np.float32))

    nc = _build_nc()
    in_maps = [
        {"Xm": Xm[k], "Xe": Xe[k], "Wt": Wtop, "Wb": Wbot, "We": Wedge}
        for k in range(N_CORES)
    ]
    res = run_bass_kernel_spmd(nc, in_maps, core_ids=list(range(N_CORES)))
    LAST_RESULT = res
    return _assemble_output(res)
